# revision 1
# baseline (speedup 1.0000x reference)
"""Trainium2 Bass kernel for nn_Conv_8443905704574.

Reference semantics: 7x7 cross-correlation (stride 1, zero pad 3) applied to
the LAST input channel only; the single-channel result is broadcast to all 3
output channels.

Device algorithm: banded-Toeplitz matmul conv. For each 128-row input block,
the 7 kernel columns become 7 stationary [128,128] band matrices (entries
T[k,m] = K[k-m+off, dj]); each is matmul'd (fp32r, full PE rate) against a
W-shifted slice of the block, accumulating the 7 taps in PSUM. One block
yields 122 valid output rows. The W zero-padding is baked into the host-side
input layout so each input block needs exactly one DMA (keeps per-matmul sync
wait counts within the ISA limit).

Sharding: pure data parallel — 2 images per core across 8 cores; host slices
the last channel, device computes [2,1024,1024], host broadcasts channels.
"""

import numpy as np

import concourse.bacc as bacc
import concourse.mybir as mybir
import concourse.tile as tile
from concourse.bass_utils import run_bass_kernel_spmd

B, C, H, W = 16, 3, 1024, 1024
KS = 7
PAD = KS // 2
NCORES = 8
PER = B // NCORES          # images per core
TILE_OUT = 128 - (KS - 1)  # 122 valid output rows per H-tile
NT = (H + TILE_OUT - 1) // TILE_OUT  # 9
WCH = 512                  # W chunk = one fp32 PSUM bank
NWCH = W // WCH            # 2
XW = W + 2 * PAD           # host-padded input width
NXB = 5                    # input block buffers

f32 = mybir.dt.float32
f32r = mybir.dt.float32r

_CACHE = {}
LAST_RESULTS = None


def _build_bass():
    nc = bacc.Bacc("TRN2", target_bir_lowering=False, debug=False)
    x = nc.dram_tensor("x", [PER, H, XW], f32r, kind="ExternalInput")
    tmat = nc.dram_tensor("tmat", [128, 2 * KS * 128], f32r, kind="ExternalInput")
    out = nc.dram_tensor("out", [PER, H, W], f32, kind="ExternalOutput")

    with tile.TileContext(nc) as tc:
        with (
            tc.tile_pool(name="tmp", bufs=1) as tmpool,
            tc.tile_pool(name="xp", bufs=1) as xpool,
            tc.tile_pool(name="op", bufs=6) as opool,
            tc.tile_pool(name="pp", bufs=6, space="PSUM") as ppool,
            tc.tile_pool(name="wz", bufs=1) as wzpool,
        ):
            x_tiles = []
            for i in range(NXB):
                xt = xpool.tile([128, XW], f32r, name=f"xt{i}", tag=f"xt{i}")
                x_tiles.append(xt)

            # PE warm-up during the DMA lead-in: zero matmuls keep the PE HAM
            # clock gate busy so real matmuls start at full clock.
            wz = wzpool.tile([128, 128 + WCH], f32, name="wz")
            nc.vector.memset(wz[:], 0.0)
            pz = [
                ppool.tile([128, WCH], f32, name=f"pz{i}", tag=f"pz{i}", bufs=1)
                for i in range(2)
            ]
            for i in range(16):
                nc.tensor.matmul(
                    pz[i % 2][:],
                    wz[:, 0:128].bitcast(f32r),
                    wz[:, 128 : 128 + WCH].bitcast(f32r),
                    start=True, stop=True,
                )

            # first input block + band matrices: the critical path
            t_sb = tmpool.tile([128, 2 * KS * 128], f32r, name="t_sb")

            def tile_geo(img, t):
                r0 = t * TILE_OUT
                nv = min(TILE_OUT, H - r0)
                # First block starts at the image edge (band offset PAD);
                # interior blocks start PAD rows above their outputs.
                if t == 0:
                    in0, variant = 0, 0
                else:
                    in0, variant = r0 - PAD, 1
                nk = min(128, H - in0)
                return r0, nv, in0, nk, variant

            schedule = [(img, t) for img in range(PER) for t in range(NT)]

            # x0 ahead of the band matrices (both gate the first real matmul)
            nc.sync.dma_start(x_tiles[0][0:128, :], x[0, 0:128, :])
            nc.sync.dma_start(t_sb[:], tmat[:])

            for idx, (img, t) in enumerate(schedule):
                r0, nv, in0, nk, variant = tile_geo(img, t)
                xt = x_tiles[idx % NXB]
                if idx > 0:
                    nc.sync.dma_start(xt[0:nk, :], x[img, in0 : in0 + nk, :])
                for c in range(NWCH):
                    pt = ppool.tile([128, WCH], f32, name="pt", tag="pt")
                    for dj in range(KS):
                        col = (variant * KS + dj) * 128
                        nc.tensor.matmul(
                            pt[:],
                            t_sb[0:nk, col : col + 128],
                            xt[0:nk, c * WCH + dj : c * WCH + dj + WCH],
                            start=(dj == 0),
                            stop=(dj == KS - 1),
                        )
                    ot = opool.tile([128, WCH], f32, name="ot", tag="ot")
                    nc.scalar.copy(ot[0:nv, :], pt[0:nv, :])
                    nc.sync.dma_start(
                        out[img, r0 : r0 + nv, c * WCH : (c + 1) * WCH],
                        ot[0:nv, :],
                    )
    nc.compile()
    return nc


def _toeplitz(kmat: np.ndarray) -> np.ndarray:
    """[128, 2*KS*128] stationary band matrices: variant 0 = first block
    (band offset PAD), variant 1 = interior blocks (band offset 0)."""
    k_idx = np.arange(128)[:, None]
    m_idx = np.arange(128)[None, :]
    t_all = np.zeros((128, 2, KS, 128), dtype=np.float32)
    for variant, off in ((0, PAD), (1, 0)):
        di = k_idx - m_idx + off
        mask = (di >= 0) & (di < KS)
        dic = np.clip(di, 0, KS - 1)
        for dj in range(KS):
            t_all[:, variant, dj, :] = np.where(mask, kmat[dic, dj], 0.0)
    return t_all.reshape(128, 2 * KS * 128)


def _shard_inputs(image: np.ndarray, kmat: np.ndarray):
    tmat = _toeplitz(kmat)
    xs = np.zeros((NCORES, PER, H, XW), dtype=np.float32)
    xs[:, :, :, PAD : PAD + W] = image[:, C - 1, :, :].reshape(
        NCORES, PER, H, W
    )
    return [{"x": xs[i], "tmat": tmat} for i in range(NCORES)]


def kernel(**inputs):
    global LAST_RESULTS
    image = np.asarray(inputs["image"], dtype=np.float32)
    kmat = np.asarray(inputs["kernel"], dtype=np.float32)
    assert image.shape == (B, C, H, W), image.shape

    if "nc" not in _CACHE:
        _CACHE["nc"] = _build_bass()
    nc = _CACHE["nc"]

    in_maps = _shard_inputs(image, kmat)
    res = run_bass_kernel_spmd(nc, in_maps, list(range(NCORES)))
    LAST_RESULTS = res

    y = np.stack([res.results[i]["out"] for i in range(NCORES)], axis=0)
    y = y.reshape(B, 1, H, W)
    return np.broadcast_to(y, (B, C, H, W))



# revision 2
# speedup vs baseline: 57117.2246x; 57117.2246x over previous
"""Trainium2 Bass kernel for nn_Conv_8443905704574.

Reference semantics: 7x7 cross-correlation (stride 1, zero pad 3) applied to
the LAST input channel only; the single-channel result is broadcast to all 3
output channels.

Device algorithm: banded-Toeplitz matmul conv in bf16. Host zero-pads the
image (H and W) and packs it per core as x[128, 18, 1030]: partition p of
block b holds padded image row 122*b + p, so every DMA spans all 128 SBUF
partitions (the HWDGE sprays descriptors across all 16 SDMA engines only for
128-partition transfers; partial-partition stores collapse onto 2 engines at
~45 GB/s, which was the old bottleneck). For each 128-row block the 7 kernel
columns become 7 stationary [128,128] bf16 band matrices (T[k,m] = K[k-m,dj]);
each is matmul'd against a W-shifted slice of the block, accumulating in
PSUM (fp32). 122 of 128 output rows are valid. Results are cast to bf16 by
the scalar engine and staged as y[128, 18, 1024] in DRAM; the host unpacks
the valid rows and upcasts to fp32.

Sharding: pure data parallel - 2 images per core across 8 cores.
"""

import numpy as np
import ml_dtypes

import concourse.bacc as bacc
import concourse.mybir as mybir
import concourse.tile as tile
from concourse.bass_utils import run_bass_kernel_spmd

B, C, H, W = 16, 3, 1024, 1024
KS = 7
PAD = KS // 2
NCORES = 8
PER = B // NCORES          # images per core
TILE_OUT = 128 - (KS - 1)  # 122 valid output rows per block
NBI = 9                    # blocks per image
NBLK = PER * NBI           # blocks per core
XW = W + 2 * PAD           # host-padded input width (1030)
HP = TILE_OUT * (NBI - 1) + 128  # host-padded input height (1104)
LG = 6                     # blocks per input DMA
SG = 3                     # blocks per output DMA

f32 = mybir.dt.float32
bf16 = mybir.dt.bfloat16

_CACHE = {}
LAST_RESULTS = None


def _build_bass():
    nc = bacc.Bacc("TRN2", target_bir_lowering=False, debug=False)
    x = nc.dram_tensor("x", [128, NBLK, XW], bf16, kind="ExternalInput")
    tmat = nc.dram_tensor("tmat", [128, KS * 128], bf16, kind="ExternalInput")
    y = nc.dram_tensor("y", [128, NBLK, W], bf16, kind="ExternalOutput")

    with tile.TileContext(nc) as tc:
        with (
            tc.tile_pool(name="xp", bufs=2) as xpool,
            tc.tile_pool(name="tp", bufs=1) as tpool,
            tc.tile_pool(name="op", bufs=2) as opool,
            tc.tile_pool(name="pp", bufs=6, space="PSUM") as ppool,
            tc.tile_pool(name="wp", bufs=1) as wzpool,
        ):
            # PE warm-up: zero matmuls keep the HAM clock gate busy so the
            # real matmuls run at 2.4 GHz from the start.
            wz = wzpool.tile([128, 128 + 512], bf16, name="wz")
            nc.vector.memset(wz[:], 0.0)
            pz = [
                ppool.tile([128, 512], f32, name=f"pz{i}", tag=f"pz{i}", bufs=1)
                for i in range(2)
            ]
            for i in range(18):
                nc.tensor.matmul(
                    pz[i % 2][:],
                    wz[:, 0:128],
                    wz[:, 128 : 128 + 512],
                    start=True,
                    stop=True,
                )

            ts = tpool.tile([128, KS * 128], bf16, name="ts")
            nc.sync.dma_start(ts[:], tmat[:])

            for g in range(NBLK // LG):
                xg = xpool.tile([128, LG * XW], bf16, name="xg", tag="xg")
                nc.sync.dma_start(xg[:], x[:, g * LG : (g + 1) * LG, :])
                for sg in range(LG // SG):
                    ot = opool.tile([128, SG * W], bf16, name="ot", tag="ot")
                    for j3 in range(SG):
                        j = sg * SG + j3
                        for c in range(2):
                            pt = ppool.tile([128, 512], f32, name="pt", tag="pt")
                            base = j * XW + c * 512
                            for dj in range(KS):
                                nc.tensor.matmul(
                                    pt[:],
                                    ts[:, dj * 128 : (dj + 1) * 128],
                                    xg[:, base + dj : base + dj + 512],
                                    start=(dj == 0),
                                    stop=(dj == KS - 1),
                                )
                            nc.scalar.copy(
                                ot[:, j3 * W + c * 512 : j3 * W + c * 512 + 512],
                                pt[:],
                            )
                    b0 = g * LG + sg * SG
                    nc.sync.dma_start(y[:, b0 : b0 + SG, :], ot[:])
    nc.compile()
    return nc


def _toeplitz(kmat: np.ndarray) -> np.ndarray:
    """[128, KS*128] stationary band matrices, bf16: T[:, dj*128+m] column m
    maps output row m to input rows m..m+6 with weights K[:, dj]."""
    k_idx = np.arange(128)[:, None]
    m_idx = np.arange(128)[None, :]
    di = k_idx - m_idx
    mask = (di >= 0) & (di < KS)
    dic = np.clip(di, 0, KS - 1)
    t = np.zeros((128, KS, 128), dtype=np.float32)
    for dj in range(KS):
        t[:, dj, :] = np.where(mask, kmat[dic, dj], 0.0)
    return t.reshape(128, KS * 128).astype(ml_dtypes.bfloat16)


def _shard_inputs(image: np.ndarray, kmat: np.ndarray):
    tmat = _toeplitz(kmat)
    xb = image[:, C - 1].astype(ml_dtypes.bfloat16)  # [B, H, W]
    pad = np.zeros((B, HP, XW), dtype=ml_dtypes.bfloat16)
    pad[:, PAD : PAD + H, PAD : PAD + W] = xb
    idx = TILE_OUT * np.arange(NBI)[None, :] + np.arange(128)[:, None]  # [128, NBI]
    xg = pad[:, idx, :]  # [B, 128, NBI, XW]
    xg = (
        xg.reshape(NCORES, PER, 128, NBI, XW)
        .transpose(0, 2, 1, 3, 4)
        .reshape(NCORES, 128, NBLK, XW)
    )
    return [
        {"x": np.ascontiguousarray(xg[i]), "tmat": tmat} for i in range(NCORES)
    ]


def _unpack_output(results) -> np.ndarray:
    y = np.empty((B, H, W), dtype=np.float32)
    for i in range(NCORES):
        yf = np.asarray(results[i]["y"]).astype(np.float32)  # [128, NBLK, W]
        for b in range(NBLK):
            img = PER * i + b // NBI
            r0 = TILE_OUT * (b % NBI)
            nv = min(TILE_OUT, H - r0)
            y[img, r0 : r0 + nv] = yf[0:nv, b]
    return y


def kernel(**inputs):
    global LAST_RESULTS
    image = np.asarray(inputs["image"], dtype=np.float32)
    kmat = np.asarray(inputs["kernel"], dtype=np.float32)
    assert image.shape == (B, C, H, W), image.shape

    if "nc" not in _CACHE:
        _CACHE["nc"] = _build_bass()
    nc = _CACHE["nc"]

    in_maps = _shard_inputs(image, kmat)
    res = run_bass_kernel_spmd(nc, in_maps, list(range(NCORES)))
    LAST_RESULTS = res

    y = _unpack_output(res.results)
    return np.broadcast_to(y[:, None], (B, C, H, W))


# revision 5
# speedup vs baseline: 71111.5809x; 1.2450x over previous
"""Trainium2 Bass kernel for nn_Conv_8443905704574.

Reference semantics: 7x7 cross-correlation (stride 1, zero pad 3) applied to
the LAST input channel only; the single-channel result is broadcast to all 3
output channels.

Device algorithm: banded-Toeplitz matmul conv in bf16 using 32x32 PE-array
tiling. The 128x128 PE array is addressed as 16 independent 32x32 tiles
(tile_position=(32*ki, 32*mj)); 16 matmuls issue back-to-back and execute
concurrently on the sub-arrays (~436 ns per 16-MM slot at N=512, vs 216 ns
for ONE full-128 matmul). Each tile convolves a 32-row window of the image
producing 26 valid output rows: the stationary is a [32,32] band matrix
(T[k,m] = K[k-m,dj]) per kernel column dj, the moving operand a W-shifted
[32,512] slice; 7 taps accumulate in fp32 PSUM. One "round" = 16 row-groups
x 2 W-chunks = 14 concurrent-16 slots; 5 rounds cover a core's 2 images.

DMA: every transfer spans all 128 SBUF partitions (the HWDGE sprays
descriptors across all 16 SDMA engines only for 128-partition transfers;
partial-partition stores collapse onto 2 engines at ~45 GB/s). Host packs
x[128, 5, 4, 1030] bf16 (partition 32*ki+q of (round r, mj) holds padded
image row 26*(16r+4mj+ki)+q) and unpacks y[128, 5, 4, 1024] bf16. PSUM banks
are drained by Scalar (ki 0-1) and Vector (ki 2-3) engines in parallel,
casting fp32->bf16.

Sharding: pure data parallel - 2 images per core across 8 cores.
"""

import numpy as np
import ml_dtypes

import concourse.bacc as bacc
import concourse.mybir as mybir
import concourse.tile as tile
from concourse.bass_utils import run_bass_kernel_spmd

B, C, H, W = 16, 3, 1024, 1024
KS = 7
PAD = KS // 2
NCORES = 8
PER = B // NCORES            # images per core
GR = 32 - (KS - 1)           # 26 valid output rows per 32-row tile window
NGI = (H + GR - 1) // GR     # 40 row-groups per image
NG = PER * NGI               # 80 row-groups per core
ROUNDS = NG // 16            # 5 rounds of 16 concurrent tiles
XW = W + 2 * PAD             # host-padded input width (1030)
HP = GR * (NGI - 1) + 32     # host-padded input height (1046)

f32 = mybir.dt.float32
bf16 = mybir.dt.bfloat16

_CACHE = {}
LAST_RESULTS = None


def _build_bass():
    nc = bacc.Bacc("TRN2", target_bir_lowering=False, debug=False)
    x = nc.dram_tensor("x", [128, ROUNDS, 4, XW], bf16, kind="ExternalInput")
    tmat = nc.dram_tensor("tmat", [128, KS * 32], bf16, kind="ExternalInput")
    y = nc.dram_tensor("y", [128, ROUNDS, 4, W], bf16, kind="ExternalOutput")

    with tile.TileContext(nc) as tc:
        with (
            tc.tile_pool(name="xp", bufs=ROUNDS) as xpool,
            tc.tile_pool(name="tp", bufs=1) as tpool,
            tc.tile_pool(name="op", bufs=2) as opool,
            tc.tile_pool(name="pp", bufs=1, space="PSUM") as ppool,
            tc.tile_pool(name="wp", bufs=1) as wzpool,
        ):
            # 8 PSUM banks: (ki, chunk) -> one [128,512] bank holding the
            # 4 mj tiles' outputs stacked along partitions.
            ps = [
                [
                    ppool.tile([128, 512], f32, name=f"ps{ki}{c}", tag=f"ps{ki}{c}")
                    for c in range(2)
                ]
                for ki in range(4)
            ]

            # Stationaries first (tiny, gates the first real matmul), then
            # all input rounds up front — bufs=ROUNDS, so no reuse hazard
            # and the sync DGE FIFO never blocks on a compute dependency.
            ts = tpool.tile([128, KS * 32], bf16, name="ts")
            nc.sync.dma_start(ts[:], tmat[:])

            xgs = []
            for r in range(ROUNDS):
                xg = xpool.tile([128, 4 * XW], bf16, name=f"xg", tag="xg")
                xgs.append(xg)
                nc.sync.dma_start(xg[:], x[:, r, :, :])

            # PE warm-up: zero matmuls release the HAM clock gate so real
            # matmuls run at 2.4 GHz. Writes land in ps banks and are
            # overwritten by the first start=True tap.
            wz = wzpool.tile([128, 128 + 512], bf16, name="wz")
            nc.vector.memset(wz[:], 0.0)
            for i in range(18):
                nc.tensor.matmul(
                    ps[i % 2][0][:],
                    wz[:, 0:128],
                    wz[:, 128 : 128 + 512],
                    start=True,
                    stop=True,
                )

            for r in range(ROUNDS):
                xg = xgs[r]
                for c in range(2):
                    for dj in range(KS):
                        for ki in range(4):
                            for mj in range(4):
                                nc.tensor.matmul(
                                    ps[ki][c][32 * mj : 32 * mj + 32, :],
                                    ts[32 * ki : 32 * ki + 32, dj * 32 : dj * 32 + 32],
                                    xg[
                                        32 * ki : 32 * ki + 32,
                                        mj * XW + c * 512 + dj : mj * XW + c * 512 + dj + 512,
                                    ],
                                    start=(dj == 0),
                                    stop=(dj == KS - 1),
                                    tile_position=(32 * ki, 32 * mj),
                                )
                ot = opool.tile([128, 4 * W], bf16, name="ot", tag="ot")
                for ki in range(4):
                    for c in range(2):
                        dst = ot[:, ki * W + c * 512 : ki * W + c * 512 + 512]
                        if ki < 2:
                            nc.scalar.copy(dst, ps[ki][c][:])
                        else:
                            nc.vector.tensor_copy(dst, ps[ki][c][:])
                nc.sync.dma_start(y[:, r, :, :], ot[:])
    nc.compile()
    return nc


def _toeplitz(kmat: np.ndarray) -> np.ndarray:
    """[128, KS*32] bf16: four identical [32, KS*32] stationary band-matrix
    strips (one per PE row-group). T[k, dj*32+m] = K[k-m, dj] for k-m in
    [0, KS)."""
    k_idx = np.arange(32)[:, None]
    m_idx = np.arange(32)[None, :]
    di = k_idx - m_idx
    mask = (di >= 0) & (di < KS)
    dic = np.clip(di, 0, KS - 1)
    t = np.zeros((32, KS, 32), dtype=np.float32)
    for dj in range(KS):
        t[:, dj, :] = np.where(mask, kmat[dic, dj], 0.0)
    t = t.reshape(32, KS * 32)
    return np.tile(t, (4, 1)).astype(ml_dtypes.bfloat16)


def _shard_inputs(image: np.ndarray, kmat: np.ndarray):
    tmat = _toeplitz(kmat)
    xb = image[:, C - 1].astype(ml_dtypes.bfloat16)  # [B, H, W]
    pad = np.zeros((B, HP, XW), dtype=ml_dtypes.bfloat16)
    pad[:, PAD : PAD + H, PAD : PAD + W] = xb

    p = np.arange(128)
    ki = (p >> 5)[:, None, None]                      # [128,1,1]
    q = (p & 31)[:, None, None]
    r = np.arange(ROUNDS)[None, :, None]              # [1,R,1]
    mj = np.arange(4)[None, None, :]                  # [1,1,4]
    g = 16 * r + 4 * mj + ki                          # [128,R,4] core-group id
    img_loc = g // NGI
    row = GR * (g % NGI) + q                          # [128,R,4]

    in_maps = []
    for i in range(NCORES):
        xi = pad[2 * i + img_loc, row, :]             # [128,R,4,XW]
        in_maps.append({"x": np.ascontiguousarray(xi), "tmat": tmat})
    return in_maps


def _unpack_output(results) -> np.ndarray:
    y = np.empty((B, H, W), dtype=np.float32)
    for i in range(NCORES):
        arr = np.asarray(results[i]["y"]).astype(np.float32)  # [128,R,4,W]
        for r in range(ROUNDS):
            for mj in range(4):
                for ki in range(4):
                    g = 16 * r + 4 * mj + ki
                    img = PER * i + g // NGI
                    r0 = GR * (g % NGI)
                    nv = min(GR, H - r0)
                    y[img, r0 : r0 + nv] = arr[32 * mj : 32 * mj + nv, r, ki]
    return y


def kernel(**inputs):
    global LAST_RESULTS
    image = np.asarray(inputs["image"], dtype=np.float32)
    kmat = np.asarray(inputs["kernel"], dtype=np.float32)
    assert image.shape == (B, C, H, W), image.shape

    if "nc" not in _CACHE:
        _CACHE["nc"] = _build_bass()
    nc = _CACHE["nc"]

    in_maps = _shard_inputs(image, kmat)
    res = run_bass_kernel_spmd(nc, in_maps, list(range(NCORES)))
    LAST_RESULTS = res

    y = _unpack_output(res.results)
    return np.broadcast_to(y[:, None], (B, C, H, W))
